# revision 52
# baseline (speedup 1.0000x reference)
"""DiGCNNet forward on 8 Trainium2 NeuronCores, data-parallel over batch.

Math (per batch b):
  adj = mean_t graph_sigs[b]                  # [30, 30]
  xw  = real[b] @ W                           # [30, 256]
  agg = adj^T @ xw + conv_bias                # [30, 256]
  h   = relu(agg)
  ns  = h @ pool_w + pool_b                   # [30]
  lg  = ns @ head_w^T + head_b                # [7]
  out = softmax(lg)

Device strategy per core (64 batches, 16 groups of 4, bf16 inputs):
  - gs cast to bf16 on host, stored [4096, 900] (rows = (b, t)).  Loaded in 8
    chunks of 512 rows with partition p <- row 4p+c ("(p c) m"), giving 7.2KB
    contiguous DMA descriptors.  Batch b of a chunk occupies partitions
    [16b, 16b+16) for every c, so a block-structured ones8 [128, 8] matmul
    (accumulated over c=0..3) T-reduces 8 batches -> PSUM [8, 900].
  - adj -> block-diagonal via a DRAM round-trip (SBUF-side DMA APs cannot
    split a free dim across partitions, but DRAM APs have arbitrary strides):
    per group scatter-write adjs[4h:4h+4] into a zero-initialized DRAM
    scratch laid out [128 rows (32k+i), 16 groups, 120 cols (30k+j)], then
    read back per chunk as one [128, 2, 120] tile.  Pad rows stay zero.
  - xw: realT padded to [512, 2048] bf16 on host (group g at cols 128g+32k+n,
    zeros at n=30,31); 4 accumulating matmuls vs W chunks -> PSUM [128, 256].
  - agg: bias matmul (ones x conv_bias) + ONE block-diag matmul
    bd[128, 120]^T @ xwb[128, 256] -> PSUM [120, 256] (rows 30k+j).
  - relu + pool-weight mult + free-dim sum fused in one DVE
    scalar_tensor_tensor (accum_out) -> node scores ns [120, 1].
  - head: [120, 28] matmul -> logits accumulated directly in a PSUM [28, 16]
    tile (one column per group); pool_b/head_b folded into the softmax exp
    bias; softmax tail as 7-block partition sums via tiny matmuls.
"""

from contextlib import ExitStack

import ml_dtypes
import numpy as np

import concourse.bacc as bacc
import concourse.bass as bass
import concourse.tile as tile
from concourse import mybir
from concourse.bass_utils import run_bass_kernel_spmd

F32 = mybir.dt.float32
BF16 = mybir.dt.bfloat16
FP8 = mybir.dt.float8e4
NP_BF16 = ml_dtypes.bfloat16
NP_FP8 = ml_dtypes.float8_e4m3

B, T, N = 512, 64, 30
F_IN, D, C = 512, 256, 7
NCORES = 8
BL = B // NCORES        # 64 batches per core
GPB = 4                 # batches per group
NG = BL // GPB          # 16 groups
NN = N * N              # 900
NB = GPB * N            # 120 packed rows per group
PB = 32                 # padded rows per batch block
NCHUNK = 8              # gs chunks; each = 512 rows = 8 batches = 2 groups
BPC = 8                 # batches per chunk
BDW = NG * NB           # 1920: bd scratch row width


def _build_nc():
    nc = bacc.Bacc(None, target_bir_lowering=False)

    gs8 = nc.dram_tensor("gs8", (BL * T, NN), FP8, kind="ExternalInput")
    ones16 = nc.dram_tensor("ones16", (128, 2, 16), FP8, kind="ExternalInput")
    rtp = nc.dram_tensor("rtp", (F_IN, NG * 128), BF16, kind="ExternalInput")
    wt = nc.dram_tensor("wt", (128, 4, D), BF16, kind="ExternalInput")
    # consolidated constants: one bf16 blob, one f32 blob (single DMA each)
    # cbh[0, 8:136] = ones1; cbh[0, 136:648] = conv_bias x2
    cbh = nc.dram_tensor("cbh", (128, 648), BF16, kind="ExternalInput")
    # cf[:, 0:256] = pwb; cf[0:120, 256:284] = hwblk; cf[0:28, 284] = hbb;
    # cf[0:28, 285:289] = b7; cf[0:4, 289:317] = b7t
    cf = nc.dram_tensor("cf", (128, 320), F32, kind="ExternalInput")
    # block-diag scratch: element (32k+i)*1920 + g*120 + 30k+j
    bdram = nc.dram_tensor("bdram", (128, BDW), BF16, kind="Internal")
    out = nc.dram_tensor("out", (BL, C), F32, kind="ExternalOutput")

    with tile.TileContext(nc) as tc, ExitStack() as ctx:
        consts = ctx.enter_context(tc.tile_pool(name="consts", bufs=1))
        gs_pool = ctx.enter_context(tc.tile_pool(name="gsp", bufs=NCHUNK))
        adjs_pool = ctx.enter_context(tc.tile_pool(name="adjs", bufs=4))
        bd_pool = ctx.enter_context(tc.tile_pool(name="bd", bufs=4))
        xwb_pool = ctx.enter_context(tc.tile_pool(name="xwb", bufs=NCHUNK))
        scr_pool = ctx.enter_context(tc.tile_pool(name="scr", bufs=2))
        ns_pool = ctx.enter_context(tc.tile_pool(name="ns", bufs=4))
        tail_pool = ctx.enter_context(tc.tile_pool(name="tail", bufs=1))
        adjp_pool = ctx.enter_context(
            tc.tile_pool(name="adjp", bufs=2, space=bass.MemorySpace.PSUM)
        )
        xwp_pool = ctx.enter_context(
            tc.tile_pool(name="xwp", bufs=1, space=bass.MemorySpace.PSUM)
        )
        aggp_pool = ctx.enter_context(
            tc.tile_pool(name="aggp", bufs=2, space=bass.MemorySpace.PSUM)
        )
        lgp_pool = ctx.enter_context(
            tc.tile_pool(name="lgp", bufs=1, space=bass.MemorySpace.PSUM)
        )

        # consts + weights on the scalar queue (gs stream owns sync)
        ones16_sb = consts.tile([128, 2, 16], FP8, tag="ones16")
        nc.scalar.dma_start(ones16_sb[:], ones16[:])
        cbh_sb = consts.tile([128, 648], BF16, tag="cbh")
        nc.scalar.dma_start(cbh_sb[:], cbh[:])
        cf_sb = consts.tile([128, 320], F32, tag="cf")
        nc.scalar.dma_start(cf_sb[:], cf[:])
        ones1_sb = cbh_sb[0:1, BPC : BPC + NB]
        cb2_sb = cbh_sb[0:1, 136 : 136 + 2 * D]
        pwb_sb = cf_sb[0:NB, 0:D]
        hw_sb = cf_sb[0:NB, D : D + GPB * C]
        hbb_sb = cf_sb[0 : GPB * C, 284:285]
        b7_sb = cf_sb[0 : GPB * C, 285:289]
        b7t_sb = cf_sb[0:GPB, 289:317]

        wt_sb = consts.tile([128, 4, D], BF16, tag="wt")
        nc.scalar.dma_start(wt_sb[:], wt[:])
        rt_all = consts.tile([128, 4, NG * 128], BF16, tag="rt_all")
        for h4 in range(2):
            cs = h4 * 512
            nc.scalar.dma_start(
                rt_all[:, :, cs : cs + 512],
                rtp[:, cs : cs + 512].rearrange("(p c) m -> p c m", c=4),
            )

        # zero-init the bd scratch (one contiguous write)
        zt = consts.tile([128, BDW], BF16, tag="zt")
        nc.vector.memset(zt[:], 0.0)
        nc.scalar.dma_start(bdram[:], zt[:])

        # gs stream: 8 chunks of [128, 4, 900] fp8 on the sync queue, each in
        # two half-DMAs so the first T-reduce matmul starts half a chunk early;
        # rt halves 2,3 ride behind it (needed only by groups 8+)
        gs_tiles = []
        for s in range(NCHUNK):
            gt = gs_pool.tile([128, 4, NN], FP8, tag="gt", name=f"gt{s}")
            src = gs8[512 * s : 512 * (s + 1)].rearrange("(p c) m -> p c m", c=4)
            nc.sync.dma_start(gt[:, 0:2, :], src[:, 0:2, :])
            nc.sync.dma_start(gt[:, 2:4, :], src[:, 2:4, :])
            gs_tiles.append(gt)
        for h4 in range(2, 4):
            cs = h4 * 512
            nc.sync.dma_start(
                rt_all[:, :, cs : cs + 512],
                rtp[:, cs : cs + 512].rearrange("(p c) m -> p c m", c=4),
            )

        logits_p = lgp_pool.tile([GPB * C, NG], F32, tag="logits")
        bd_tiles = {}

        def emit_treduce(s):
            # fp8 DoubleRow: K=256 per matmul (two c-columns), each batch
            # duplicated on 2 out rows (M=16; M=8 trips lw_dual_fp8 ISA check)
            adjp_t = adjp_pool.tile([2 * BPC, NN], F32, tag="adjp")
            for h in range(2):
                for lo, hi in ((0, 512), (512, NN)):
                    nc.tensor.matmul(
                        adjp_t[:, lo:hi], ones16_sb[:],
                        gs_tiles[s][:, 2 * h : 2 * h + 2, lo:hi],
                        start=(h == 0), stop=(h == 1),
                        perf_mode=mybir.MatmulPerfMode.DoubleRow,
                    )
            adjs_t = adjs_pool.tile([2 * BPC, NN], BF16, tag="adjs")
            nc.vector.tensor_copy(adjs_t[:, 0:300], adjp_t[:, 0:300])
            nc.scalar.copy(adjs_t[:, 300:NN], adjp_t[:, 300:NN])
            # scatter-write the two groups' diag blocks into the DRAM scratch
            adjs_ev = adjs_t[:].rearrange("(b r) m -> b r m", r=2)
            for half, eng in ((0, nc.gpsimd), (1, nc.scalar)):
                g = 2 * s + half
                wdst = bass.AP(
                    bdram, NB * g, [[PB * BDW + N, GPB], [BDW, N], [1, N]]
                )
                eng.dma_start(wdst, adjs_ev[4 * half : 4 * half + 4, 0, :])
            # read back both groups as one [128, 2, 120] block-diag tile
            bd_t = bd_pool.tile([128, 2, NB], BF16, tag="bd", name=f"bd{s}")
            nc.scalar.dma_start(
                bd_t[:],
                bdram[:, 2 * NB * s : 2 * NB * (s + 1)].rearrange(
                    "p (h m) -> p h m", h=2
                ),
            )
            bd_tiles[s] = bd_t

        ns_tiles = {}
        xwb_tiles = {}

        def emit_xw(s):
            # xw depends only on rt/wt -- hoisted early; xwb persists per pair
            xwp_t = xwp_pool.tile([128, 2, D], F32, tag="xwp")
            xwb_t = xwb_pool.tile([128, 2, D], BF16, tag="xwb", name=f"xwb{s}")
            xwb_tiles[s] = xwb_t
            for h in range(2):
                g = 2 * s + h
                for c4 in range(4):
                    nc.tensor.matmul(
                        xwp_t[:, h, :], rt_all[:, c4, 128 * g : 128 * (g + 1)],
                        wt_sb[:, c4, :], start=(c4 == 0), stop=(c4 == 3),
                    )
                nc.vector.tensor_copy(xwb_t[:, h, :], xwp_t[:, h, :])

        def emit_agg(s):
            # groups 2s, 2s+1 share one PSUM pair tile
            aggp_t = aggp_pool.tile([NB, 2, D], F32, tag="aggp")
            ns2_t = ns_pool.tile([NB, 2], F32, tag="ns", name=f"ns{s}")
            ns_tiles[s] = ns2_t
            nc.tensor.matmul(
                aggp_t[:], ones1_sb[:], cb2_sb[:],
                start=True, stop=False, skip_group_check=True,
            )
            for h in range(2):
                nc.tensor.matmul(
                    aggp_t[:, h, :], bd_tiles[s][:, h, :], xwb_tiles[s][:, h, :],
                    start=False, stop=(h == 1), skip_group_check=True,
                )
            # relu + pool-weight mult + free-dim sum fused: one DVE pass each
            for h in range(2):
                scr_t = scr_pool.tile([NB, D], F32, tag="scr")
                nc.vector.scalar_tensor_tensor(
                    scr_t[:], aggp_t[:, h, :], 0.0, pwb_sb[:],
                    mybir.AluOpType.max, mybir.AluOpType.mult,
                    accum_out=ns2_t[:, h : h + 1],
                )

        def emit_head(s):
            nc.tensor.matmul(
                logits_p[:, 2 * s : 2 * s + 2], hw_sb[:], ns_tiles[s][:],
                start=True, stop=True,
            )

        # software-pipelined: xw hoisted (no gs dependency), bd(s) consumed
        # one chunk after its read, heads one pair later
        emit_treduce(0)
        emit_treduce(1)
        for s in range(2, NCHUNK):
            emit_treduce(s)
            emit_xw(s - 2)
            if s >= 3:
                emit_agg(s - 3)
            if s >= 4:
                emit_head(s - 4)
        emit_xw(NCHUNK - 2)
        emit_agg(NCHUNK - 3)
        emit_head(NCHUNK - 4)
        emit_xw(NCHUNK - 1)
        emit_agg(NCHUNK - 2)
        emit_head(NCHUNK - 3)
        emit_agg(NCHUNK - 1)
        emit_head(NCHUNK - 2)
        emit_head(NCHUNK - 1)

        # softmax over the 7 classes (partition sub-blocks of 7)
        e_t = tail_pool.tile([GPB * C, NG], F32, tag="e")
        nc.scalar.activation(
            e_t[:], logits_p[:], mybir.ActivationFunctionType.Exp, bias=hbb_sb
        )
        sum_p = xwp_pool.tile([GPB, NG], F32, tag="xwp", name="sum_p")
        nc.tensor.matmul(sum_p[:], b7_sb[:], e_t[:], start=True, stop=True)
        ssb_t = tail_pool.tile([GPB, NG], F32, tag="ssb")
        nc.vector.tensor_copy(ssb_t[:], sum_p[:])
        bcast_p = aggp_pool.tile([GPB * C, NG], F32, tag="aggp", name="bcast_p")
        nc.tensor.matmul(bcast_p[:], b7t_sb[:], ssb_t[:], start=True, stop=True)
        rs_t = tail_pool.tile([GPB * C, NG], F32, tag="rs")
        nc.vector.reciprocal(rs_t[:], bcast_p[:])
        res_t = tail_pool.tile([GPB * C, NG], F32, tag="res")
        nc.vector.tensor_mul(res_t[:], e_t[:], rs_t[:])
        nc.sync.dma_start(out.rearrange("(g bi) c -> (bi c) g", bi=GPB), res_t[:])

    nc.compile()
    return nc


_NC_CACHE = None


def _get_nc():
    global _NC_CACHE
    if _NC_CACHE is None:
        _NC_CACHE = _build_nc()
    return _NC_CACHE


def _f32c(x):
    return np.ascontiguousarray(np.asarray(x, dtype=np.float32))


def _prepare_in_maps(real, graph_sigs, W, conv_bias, pool_w, pool_b, head_w, head_b):
    real = _f32c(real)
    graph_sigs = _f32c(graph_sigs)
    W = _f32c(W)
    head_w = _f32c(head_w)

    wt = W.reshape(128, 4, D).astype(NP_BF16)  # wt[p, c, :] = W[4p+c]
    # fp8 DoubleRow T-reduce weights: batch m//2 at partitions [16b, 16b+16)
    ones16 = np.zeros((128, 2, 16), dtype=np.float32)
    for m in range(16):
        b = m // 2
        ones16[16 * b : 16 * (b + 1), :, m] = np.float32(1.0 / T)
    ones16 = ones16.astype(NP_FP8)
    # bf16 blob: ones1 | conv_bias x2
    cbh = np.zeros((128, 648), dtype=NP_BF16)
    cbh[0, BPC : BPC + NB] = np.float32(1.0)
    cbh[0, 136 : 136 + D] = _f32c(conv_bias).astype(NP_BF16)
    cbh[0, 136 + D : 136 + 2 * D] = _f32c(conv_bias).astype(NP_BF16)
    # f32 blob: pwb | hwblk | hbb | b7 | b7t
    cf = np.zeros((128, 320), dtype=np.float32)
    cf[:, 0:D] = _f32c(pool_w)[None, :]
    # hwblk rows 30k+n -> col k*7+c = head_w[c, n]
    for k in range(GPB):
        cf[N * k : N * (k + 1), D + C * k : D + C * (k + 1)] = head_w.T
    # pool_b shifts every node score by a constant; fold into the head bias
    hb_eff = _f32c(head_b) + np.float32(np.asarray(pool_b)) * head_w.sum(axis=1)
    cf[0 : GPB * C, 284] = np.tile(hb_eff, GPB)
    for k in range(GPB):
        cf[C * k : C * (k + 1), 285 + k] = 1.0
    for k in range(GPB):
        cf[0:GPB, 289 + C * k : 289 + C * (k + 1)] = np.eye(GPB)[:, k : k + 1]

    consts = {"wt": wt, "cbh": cbh, "cf": cf, "ones16": ones16}
    in_maps = []
    for c in range(NCORES):
        s = slice(c * BL, (c + 1) * BL)
        gs8 = graph_sigs[s].reshape(BL * T, NN).astype(NP_FP8)
        rt = real[s].transpose(2, 0, 1).reshape(F_IN, NG, GPB, N)
        rtp = np.zeros((F_IN, NG, GPB, PB), dtype=NP_BF16)
        rtp[:, :, :, 0:N] = rt.astype(NP_BF16)
        in_maps.append(
            {
                "gs8": gs8,
                "rtp": np.ascontiguousarray(rtp.reshape(F_IN, NG * 128)),
                **consts,
            }
        )
    return in_maps


def kernel(real, imag, graph_sigs, W, conv_bias, pool_w, pool_b, head_w, head_b):
    del imag  # unused by the forward pass
    in_maps = _prepare_in_maps(
        real, graph_sigs, W, conv_bias, pool_w, pool_b, head_w, head_b
    )
    nc = _get_nc()
    res = run_bass_kernel_spmd(nc, in_maps, core_ids=list(range(NCORES)))
    return np.concatenate([res.results[c]["out"] for c in range(NCORES)], axis=0)


# revision 54
# speedup vs baseline: 1.0018x; 1.0018x over previous
"""DiGCNNet forward on 8 Trainium2 NeuronCores, data-parallel over batch.

Math (per batch b):
  adj = mean_t graph_sigs[b]                  # [30, 30]
  xw  = real[b] @ W                           # [30, 256]
  agg = adj^T @ xw + conv_bias                # [30, 256]
  h   = relu(agg)
  ns  = h @ pool_w + pool_b                   # [30]
  lg  = ns @ head_w^T + head_b                # [7]
  out = softmax(lg)

Device strategy per core (64 batches, 16 groups of 4, bf16 inputs):
  - gs cast to bf16 on host, stored [4096, 900] (rows = (b, t)).  Loaded in 8
    chunks of 512 rows with partition p <- row 4p+c ("(p c) m"), giving 7.2KB
    contiguous DMA descriptors.  Batch b of a chunk occupies partitions
    [16b, 16b+16) for every c, so a block-structured ones8 [128, 8] matmul
    (accumulated over c=0..3) T-reduces 8 batches -> PSUM [8, 900].
  - adj -> block-diagonal via a DRAM round-trip (SBUF-side DMA APs cannot
    split a free dim across partitions, but DRAM APs have arbitrary strides):
    per group scatter-write adjs[4h:4h+4] into a zero-initialized DRAM
    scratch laid out [128 rows (32k+i), 16 groups, 120 cols (30k+j)], then
    read back per chunk as one [128, 2, 120] tile.  Pad rows stay zero.
  - xw: realT padded to [512, 2048] bf16 on host (group g at cols 128g+32k+n,
    zeros at n=30,31); 4 accumulating matmuls vs W chunks -> PSUM [128, 256].
  - agg: bias matmul (ones x conv_bias) + ONE block-diag matmul
    bd[128, 120]^T @ xwb[128, 256] -> PSUM [120, 256] (rows 30k+j).
  - relu + pool-weight mult + free-dim sum fused in one DVE
    scalar_tensor_tensor (accum_out) -> node scores ns [120, 1].
  - head: [120, 28] matmul -> logits accumulated directly in a PSUM [28, 16]
    tile (one column per group); pool_b/head_b folded into the softmax exp
    bias; softmax tail as 7-block partition sums via tiny matmuls.
"""

from contextlib import ExitStack

import ml_dtypes
import numpy as np

import concourse.bacc as bacc
import concourse.bass as bass
import concourse.tile as tile
from concourse import mybir
from concourse.bass_utils import run_bass_kernel_spmd

F32 = mybir.dt.float32
BF16 = mybir.dt.bfloat16
FP8 = mybir.dt.float8e4
NP_BF16 = ml_dtypes.bfloat16
NP_FP8 = ml_dtypes.float8_e4m3

B, T, N = 512, 64, 30
F_IN, D, C = 512, 256, 7
NCORES = 8
BL = B // NCORES        # 64 batches per core
GPB = 4                 # batches per group
NG = BL // GPB          # 16 groups
NN = N * N              # 900
NB = GPB * N            # 120 packed rows per group
PB = 32                 # padded rows per batch block
NCHUNK = 8              # gs chunks; each = 512 rows = 8 batches = 2 groups
BPC = 8                 # batches per chunk
BDW = NG * NB           # 1920: bd scratch row width


def _build_nc():
    nc = bacc.Bacc(None, target_bir_lowering=False)

    gs8 = nc.dram_tensor("gs8", (BL * T, NN), FP8, kind="ExternalInput")
    ones16 = nc.dram_tensor("ones16", (128, 2, 16), FP8, kind="ExternalInput")
    rtp = nc.dram_tensor("rtp", (F_IN, NG * 128), BF16, kind="ExternalInput")
    wt = nc.dram_tensor("wt", (128, 4, D), BF16, kind="ExternalInput")
    # consolidated constants: one bf16 blob, one f32 blob (single DMA each)
    # cbh[0, 8:136] = ones1; cbh[0, 136:648] = conv_bias x2
    cbh = nc.dram_tensor("cbh", (128, 648), BF16, kind="ExternalInput")
    # cf[:, 0:256] = pwb; cf[0:120, 256:284] = hwblk; cf[0:28, 284] = hbb;
    # cf[0:28, 285:289] = b7; cf[0:4, 289:317] = b7t
    cf = nc.dram_tensor("cf", (128, 320), F32, kind="ExternalInput")
    # block-diag scratch: element (32k+i)*1920 + g*120 + 30k+j
    bdram = nc.dram_tensor("bdram", (128, BDW), BF16, kind="Internal")
    out = nc.dram_tensor("out", (BL, C), F32, kind="ExternalOutput")

    with tile.TileContext(nc) as tc, ExitStack() as ctx:
        consts = ctx.enter_context(tc.tile_pool(name="consts", bufs=1))
        gs_pool = ctx.enter_context(tc.tile_pool(name="gsp", bufs=NCHUNK))
        adjs_pool = ctx.enter_context(tc.tile_pool(name="adjs", bufs=4))
        bd_pool = ctx.enter_context(tc.tile_pool(name="bd", bufs=4))
        xwb_pool = ctx.enter_context(tc.tile_pool(name="xwb", bufs=NCHUNK))
        scr_pool = ctx.enter_context(tc.tile_pool(name="scr", bufs=4))
        ns_pool = ctx.enter_context(tc.tile_pool(name="ns", bufs=4))
        tail_pool = ctx.enter_context(tc.tile_pool(name="tail", bufs=1))
        adjp_pool = ctx.enter_context(
            tc.tile_pool(name="adjp", bufs=2, space=bass.MemorySpace.PSUM)
        )
        xwp_pool = ctx.enter_context(
            tc.tile_pool(name="xwp", bufs=1, space=bass.MemorySpace.PSUM)
        )
        aggp_pool = ctx.enter_context(
            tc.tile_pool(name="aggp", bufs=2, space=bass.MemorySpace.PSUM)
        )
        lgp_pool = ctx.enter_context(
            tc.tile_pool(name="lgp", bufs=1, space=bass.MemorySpace.PSUM)
        )

        # consts + weights on the scalar queue (gs stream owns sync)
        ones16_sb = consts.tile([128, 2, 16], FP8, tag="ones16")
        nc.scalar.dma_start(ones16_sb[:], ones16[:])
        cbh_sb = consts.tile([128, 648], BF16, tag="cbh")
        nc.scalar.dma_start(cbh_sb[:], cbh[:])
        cf_sb = consts.tile([128, 320], F32, tag="cf")
        nc.scalar.dma_start(cf_sb[:], cf[:])
        ones1_sb = cbh_sb[0:1, BPC : BPC + NB]
        cb2_sb = cbh_sb[0:1, 136 : 136 + 2 * D]
        pwb_sb = cf_sb[0:NB, 0:D]
        hw_sb = cf_sb[0:NB, D : D + GPB * C]
        hbb_sb = cf_sb[0 : GPB * C, 284:285]
        b7_sb = cf_sb[0 : GPB * C, 285:289]
        b7t_sb = cf_sb[0:GPB, 289:317]

        wt_sb = consts.tile([128, 4, D], BF16, tag="wt")
        nc.scalar.dma_start(wt_sb[:], wt[:])
        rt_all = consts.tile([128, 4, NG * 128], BF16, tag="rt_all")
        for h4 in range(2):
            cs = h4 * 512
            nc.scalar.dma_start(
                rt_all[:, :, cs : cs + 512],
                rtp[:, cs : cs + 512].rearrange("(p c) m -> p c m", c=4),
            )

        # zero-init the bd scratch (one contiguous write)
        zt = consts.tile([128, BDW], BF16, tag="zt")
        nc.vector.memset(zt[:], 0.0)
        nc.scalar.dma_start(bdram[:], zt[:])

        # gs stream: 8 chunks of [128, 4, 900] fp8 on the sync queue, each in
        # two half-DMAs so the first T-reduce matmul starts half a chunk early;
        # rt halves 2,3 ride behind it (needed only by groups 8+)
        gs_tiles = []
        for s in range(NCHUNK):
            gt = gs_pool.tile([128, 4, NN], FP8, tag="gt", name=f"gt{s}")
            src = gs8[512 * s : 512 * (s + 1)].rearrange("(p c) m -> p c m", c=4)
            nc.sync.dma_start(gt[:, 0:2, :], src[:, 0:2, :])
            nc.sync.dma_start(gt[:, 2:4, :], src[:, 2:4, :])
            gs_tiles.append(gt)
        for h4 in range(2, 4):
            cs = h4 * 512
            nc.sync.dma_start(
                rt_all[:, :, cs : cs + 512],
                rtp[:, cs : cs + 512].rearrange("(p c) m -> p c m", c=4),
            )

        logits_p = lgp_pool.tile([GPB * C, NG], F32, tag="logits")
        bd_tiles = {}

        def emit_treduce(s):
            # fp8 DoubleRow: K=256 per matmul (two c-columns), each batch
            # duplicated on 2 out rows (M=16; M=8 trips lw_dual_fp8 ISA check)
            adjp_t = adjp_pool.tile([2 * BPC, NN], F32, tag="adjp")
            for h in range(2):
                for lo, hi in ((0, 512), (512, NN)):
                    nc.tensor.matmul(
                        adjp_t[:, lo:hi], ones16_sb[:],
                        gs_tiles[s][:, 2 * h : 2 * h + 2, lo:hi],
                        start=(h == 0), stop=(h == 1),
                        perf_mode=mybir.MatmulPerfMode.DoubleRow,
                    )
            adjs_t = adjs_pool.tile([2 * BPC, NN], BF16, tag="adjs")
            nc.vector.tensor_copy(adjs_t[:, 0:300], adjp_t[:, 0:300])
            nc.scalar.copy(adjs_t[:, 300:NN], adjp_t[:, 300:NN])
            # scatter-write the two groups' diag blocks into the DRAM scratch
            adjs_ev = adjs_t[:].rearrange("(b r) m -> b r m", r=2)
            for half, eng in ((0, nc.gpsimd), (1, nc.scalar)):
                g = 2 * s + half
                wdst = bass.AP(
                    bdram, NB * g, [[PB * BDW + N, GPB], [BDW, N], [1, N]]
                )
                eng.dma_start(wdst, adjs_ev[4 * half : 4 * half + 4, 0, :])
            # read back both groups as one [128, 2, 120] block-diag tile
            bd_t = bd_pool.tile([128, 2, NB], BF16, tag="bd", name=f"bd{s}")
            nc.scalar.dma_start(
                bd_t[:],
                bdram[:, 2 * NB * s : 2 * NB * (s + 1)].rearrange(
                    "p (h m) -> p h m", h=2
                ),
            )
            bd_tiles[s] = bd_t

        ns_tiles = {}
        xwb_tiles = {}

        def emit_xw(s):
            # xw depends only on rt/wt -- hoisted early; xwb persists per pair.
            # PSUM->SBUF copies split across DVE/ACT for faster turnaround.
            xwp_t = xwp_pool.tile([128, 2, D], F32, tag="xwp")
            xwb_t = xwb_pool.tile([128, 2, D], BF16, tag="xwb", name=f"xwb{s}")
            xwb_tiles[s] = xwb_t
            for h, eng in ((0, nc.vector.tensor_copy), (1, nc.scalar.copy)):
                g = 2 * s + h
                for c4 in range(4):
                    nc.tensor.matmul(
                        xwp_t[:, h, :], rt_all[:, c4, 128 * g : 128 * (g + 1)],
                        wt_sb[:, c4, :], start=(c4 == 0), stop=(c4 == 3),
                    )
                eng(xwb_t[:, h, :], xwp_t[:, h, :])

        def emit_agg(s):
            # groups 2s, 2s+1 share one PSUM pair tile
            aggp_t = aggp_pool.tile([NB, 2, D], F32, tag="aggp")
            ns2_t = ns_pool.tile([NB, 2], F32, tag="ns", name=f"ns{s}")
            ns_tiles[s] = ns2_t
            nc.tensor.matmul(
                aggp_t[:], ones1_sb[:], cb2_sb[:],
                start=True, stop=False, skip_group_check=True,
            )
            for h in range(2):
                nc.tensor.matmul(
                    aggp_t[:, h, :], bd_tiles[s][:, h, :], xwb_tiles[s][:, h, :],
                    start=False, stop=(h == 1), skip_group_check=True,
                )
            # relu + pool-weight mult + free-dim sum fused: one DVE pass each
            for h in range(2):
                scr_t = scr_pool.tile([NB, D], F32, tag="scr")
                nc.vector.scalar_tensor_tensor(
                    scr_t[:], aggp_t[:, h, :], 0.0, pwb_sb[:],
                    mybir.AluOpType.max, mybir.AluOpType.mult,
                    accum_out=ns2_t[:, h : h + 1],
                )

        def emit_head(s):
            nc.tensor.matmul(
                logits_p[:, 2 * s : 2 * s + 2], hw_sb[:], ns_tiles[s][:],
                start=True, stop=True,
            )

        # software-pipelined: xw hoisted (no gs dependency), bd(s) consumed
        # one chunk after its read, heads one pair later
        emit_treduce(0)
        emit_treduce(1)
        for s in range(2, NCHUNK):
            emit_treduce(s)
            emit_xw(s - 2)
            if s >= 3:
                emit_agg(s - 3)
            if s >= 4:
                emit_head(s - 4)
        emit_xw(NCHUNK - 2)
        emit_agg(NCHUNK - 3)
        emit_head(NCHUNK - 4)
        emit_xw(NCHUNK - 1)
        emit_agg(NCHUNK - 2)
        emit_head(NCHUNK - 3)
        emit_agg(NCHUNK - 1)
        emit_head(NCHUNK - 2)
        emit_head(NCHUNK - 1)

        # softmax over the 7 classes (partition sub-blocks of 7)
        e_t = tail_pool.tile([GPB * C, NG], F32, tag="e")
        nc.scalar.activation(
            e_t[:], logits_p[:], mybir.ActivationFunctionType.Exp, bias=hbb_sb
        )
        sum_p = xwp_pool.tile([GPB, NG], F32, tag="xwp", name="sum_p")
        nc.tensor.matmul(sum_p[:], b7_sb[:], e_t[:], start=True, stop=True)
        ssb_t = tail_pool.tile([GPB, NG], F32, tag="ssb")
        nc.vector.tensor_copy(ssb_t[:], sum_p[:])
        bcast_p = aggp_pool.tile([GPB * C, NG], F32, tag="aggp", name="bcast_p")
        nc.tensor.matmul(bcast_p[:], b7t_sb[:], ssb_t[:], start=True, stop=True)
        rs_t = tail_pool.tile([GPB * C, NG], F32, tag="rs")
        nc.vector.reciprocal(rs_t[:], bcast_p[:])
        res_t = tail_pool.tile([GPB * C, NG], F32, tag="res")
        nc.vector.tensor_mul(res_t[:], e_t[:], rs_t[:])
        nc.sync.dma_start(out.rearrange("(g bi) c -> (bi c) g", bi=GPB), res_t[:])

    nc.compile()
    return nc


_NC_CACHE = None


def _get_nc():
    global _NC_CACHE
    if _NC_CACHE is None:
        _NC_CACHE = _build_nc()
    return _NC_CACHE


def _f32c(x):
    return np.ascontiguousarray(np.asarray(x, dtype=np.float32))


def _prepare_in_maps(real, graph_sigs, W, conv_bias, pool_w, pool_b, head_w, head_b):
    real = _f32c(real)
    graph_sigs = _f32c(graph_sigs)
    W = _f32c(W)
    head_w = _f32c(head_w)

    wt = W.reshape(128, 4, D).astype(NP_BF16)  # wt[p, c, :] = W[4p+c]
    # fp8 DoubleRow T-reduce weights: batch m//2 at partitions [16b, 16b+16)
    ones16 = np.zeros((128, 2, 16), dtype=np.float32)
    for m in range(16):
        b = m // 2
        ones16[16 * b : 16 * (b + 1), :, m] = np.float32(1.0 / T)
    ones16 = ones16.astype(NP_FP8)
    # bf16 blob: ones1 | conv_bias x2
    cbh = np.zeros((128, 648), dtype=NP_BF16)
    cbh[0, BPC : BPC + NB] = np.float32(1.0)
    cbh[0, 136 : 136 + D] = _f32c(conv_bias).astype(NP_BF16)
    cbh[0, 136 + D : 136 + 2 * D] = _f32c(conv_bias).astype(NP_BF16)
    # f32 blob: pwb | hwblk | hbb | b7 | b7t
    cf = np.zeros((128, 320), dtype=np.float32)
    cf[:, 0:D] = _f32c(pool_w)[None, :]
    # hwblk rows 30k+n -> col k*7+c = head_w[c, n]
    for k in range(GPB):
        cf[N * k : N * (k + 1), D + C * k : D + C * (k + 1)] = head_w.T
    # pool_b shifts every node score by a constant; fold into the head bias
    hb_eff = _f32c(head_b) + np.float32(np.asarray(pool_b)) * head_w.sum(axis=1)
    cf[0 : GPB * C, 284] = np.tile(hb_eff, GPB)
    for k in range(GPB):
        cf[C * k : C * (k + 1), 285 + k] = 1.0
    for k in range(GPB):
        cf[0:GPB, 289 + C * k : 289 + C * (k + 1)] = np.eye(GPB)[:, k : k + 1]

    consts = {"wt": wt, "cbh": cbh, "cf": cf, "ones16": ones16}
    in_maps = []
    for c in range(NCORES):
        s = slice(c * BL, (c + 1) * BL)
        gs8 = graph_sigs[s].reshape(BL * T, NN).astype(NP_FP8)
        rt = real[s].transpose(2, 0, 1).reshape(F_IN, NG, GPB, N)
        rtp = np.zeros((F_IN, NG, GPB, PB), dtype=NP_BF16)
        rtp[:, :, :, 0:N] = rt.astype(NP_BF16)
        in_maps.append(
            {
                "gs8": gs8,
                "rtp": np.ascontiguousarray(rtp.reshape(F_IN, NG * 128)),
                **consts,
            }
        )
    return in_maps


def kernel(real, imag, graph_sigs, W, conv_bias, pool_w, pool_b, head_w, head_b):
    del imag  # unused by the forward pass
    in_maps = _prepare_in_maps(
        real, graph_sigs, W, conv_bias, pool_w, pool_b, head_w, head_b
    )
    nc = _get_nc()
    res = run_bass_kernel_spmd(nc, in_maps, core_ids=list(range(NCORES)))
    return np.concatenate([res.results[c]["out"] for c in range(NCORES)], axis=0)


# revision 55
# speedup vs baseline: 1.0731x; 1.0712x over previous
"""DiGCNNet forward on 8 Trainium2 NeuronCores, data-parallel over batch.

Math (per batch b):
  adj = mean_t graph_sigs[b]                  # [30, 30]
  xw  = real[b] @ W                           # [30, 256]
  agg = adj^T @ xw + conv_bias                # [30, 256]
  h   = relu(agg)
  ns  = h @ pool_w + pool_b                   # [30]
  lg  = ns @ head_w^T + head_b                # [7]
  out = softmax(lg)

Device strategy per core (64 batches, 16 groups of 4, bf16 inputs):
  - gs cast to bf16 on host, stored [4096, 900] (rows = (b, t)).  Loaded in 8
    chunks of 512 rows with partition p <- row 4p+c ("(p c) m"), giving 7.2KB
    contiguous DMA descriptors.  Batch b of a chunk occupies partitions
    [16b, 16b+16) for every c, so a block-structured ones8 [128, 8] matmul
    (accumulated over c=0..3) T-reduces 8 batches -> PSUM [8, 900].
  - adj -> block-diagonal via a DRAM round-trip (SBUF-side DMA APs cannot
    split a free dim across partitions, but DRAM APs have arbitrary strides):
    per group scatter-write adjs[4h:4h+4] into a zero-initialized DRAM
    scratch laid out [128 rows (32k+i), 16 groups, 120 cols (30k+j)], then
    read back per chunk as one [128, 2, 120] tile.  Pad rows stay zero.
  - xw: realT padded to [512, 2048] bf16 on host (group g at cols 128g+32k+n,
    zeros at n=30,31); 4 accumulating matmuls vs W chunks -> PSUM [128, 256].
  - agg: bias matmul (ones x conv_bias) + ONE block-diag matmul
    bd[128, 120]^T @ xwb[128, 256] -> PSUM [120, 256] (rows 30k+j).
  - relu + pool-weight mult + free-dim sum fused in one DVE
    scalar_tensor_tensor (accum_out) -> node scores ns [120, 1].
  - head: [120, 28] matmul -> logits accumulated directly in a PSUM [28, 16]
    tile (one column per group); pool_b/head_b folded into the softmax exp
    bias; softmax tail as 7-block partition sums via tiny matmuls.
"""

from contextlib import ExitStack

import ml_dtypes
import numpy as np

import concourse.bacc as bacc
import concourse.bass as bass
import concourse.tile as tile
from concourse import mybir
from concourse.bass_utils import run_bass_kernel_spmd

F32 = mybir.dt.float32
BF16 = mybir.dt.bfloat16
FP8 = mybir.dt.float8e4
NP_BF16 = ml_dtypes.bfloat16
NP_FP8 = ml_dtypes.float8_e4m3

B, T, N = 512, 64, 30
F_IN, D, C = 512, 256, 7
NCORES = 8
BL = B // NCORES        # 64 batches per core
GPB = 4                 # batches per group
NG = BL // GPB          # 16 groups
NN = N * N              # 900
NB = GPB * N            # 120 packed rows per group
PB = 32                 # padded rows per batch block
NCHUNK = 8              # gs chunks; each = 512 rows = 8 batches = 2 groups
BPC = 8                 # batches per chunk
BDW = NG * NB           # 1920: bd scratch row width


def _build_nc():
    nc = bacc.Bacc(None, target_bir_lowering=False)

    gs8 = nc.dram_tensor("gs8", (BL * T, NN), FP8, kind="ExternalInput")
    ones16 = nc.dram_tensor("ones16", (128, 2, 16), FP8, kind="ExternalInput")
    rtp = nc.dram_tensor("rtp", (F_IN, NG * 128), BF16, kind="ExternalInput")
    wt = nc.dram_tensor("wt", (128, 4, D), BF16, kind="ExternalInput")
    # consolidated constants: one bf16 blob, one f32 blob (single DMA each)
    # cbh[0, 8:136] = ones1; cbh[0, 136:648] = conv_bias x2
    cbh = nc.dram_tensor("cbh", (128, 648), BF16, kind="ExternalInput")
    # cf[:, 0:256] = pwb; cf[0:120, 256:284] = hwblk; cf[0:28, 284] = hbb;
    # cf[0:28, 285:289] = b7; cf[0:4, 289:317] = b7t
    cf = nc.dram_tensor("cf", (128, 320), F32, kind="ExternalInput")
    # block-diag scratch: element (32k+i)*1920 + g*120 + 30k+j
    bdram = nc.dram_tensor("bdram", (128, BDW), BF16, kind="Internal")
    out = nc.dram_tensor("out", (BL, C), F32, kind="ExternalOutput")

    with tile.TileContext(nc) as tc, ExitStack() as ctx:
        consts = ctx.enter_context(tc.tile_pool(name="consts", bufs=1))
        gs_pool = ctx.enter_context(tc.tile_pool(name="gsp", bufs=NCHUNK))
        adjs_pool = ctx.enter_context(tc.tile_pool(name="adjs", bufs=4))
        bd_pool = ctx.enter_context(tc.tile_pool(name="bd", bufs=4))
        xwb_pool = ctx.enter_context(tc.tile_pool(name="xwb", bufs=NCHUNK))
        scr_pool = ctx.enter_context(tc.tile_pool(name="scr", bufs=4))
        ns_pool = ctx.enter_context(tc.tile_pool(name="ns", bufs=4))
        tail_pool = ctx.enter_context(tc.tile_pool(name="tail", bufs=1))
        adjp_pool = ctx.enter_context(
            tc.tile_pool(name="adjp", bufs=2, space=bass.MemorySpace.PSUM)
        )
        xwp_pool = ctx.enter_context(
            tc.tile_pool(name="xwp", bufs=1, space=bass.MemorySpace.PSUM)
        )
        aggp_pool = ctx.enter_context(
            tc.tile_pool(name="aggp", bufs=2, space=bass.MemorySpace.PSUM)
        )
        lgp_pool = ctx.enter_context(
            tc.tile_pool(name="lgp", bufs=1, space=bass.MemorySpace.PSUM)
        )

        # consts + weights on the scalar queue (gs stream owns sync)
        ones16_sb = consts.tile([128, 2, 16], FP8, tag="ones16")
        nc.scalar.dma_start(ones16_sb[:], ones16[:])
        cbh_sb = consts.tile([128, 648], BF16, tag="cbh")
        nc.scalar.dma_start(cbh_sb[:], cbh[:])
        cf_sb = consts.tile([128, 320], F32, tag="cf")
        nc.scalar.dma_start(cf_sb[:], cf[:])
        ones1_sb = cbh_sb[0:1, BPC : BPC + NB]
        cb2_sb = cbh_sb[0:1, 136 : 136 + 2 * D]
        pwb_sb = cf_sb[0:NB, 0:D]
        hw_sb = cf_sb[0:NB, D : D + GPB * C]
        hbb_sb = cf_sb[0 : GPB * C, 284:285]
        b7_sb = cf_sb[0 : GPB * C, 285:289]
        b7t_sb = cf_sb[0:GPB, 289:317]

        wt_sb = consts.tile([128, 4, D], BF16, tag="wt")
        nc.scalar.dma_start(wt_sb[:], wt[:])
        rt_all = consts.tile([128, 4, NG * 128], BF16, tag="rt_all")
        for h4 in range(2):
            cs = h4 * 512
            nc.scalar.dma_start(
                rt_all[:, :, cs : cs + 512],
                rtp[:, cs : cs + 512].rearrange("(p c) m -> p c m", c=4),
            )

        # zero-init the bd scratch (one contiguous write)
        zt = consts.tile([128, BDW], BF16, tag="zt")
        nc.vector.memset(zt[:], 0.0)
        nc.scalar.dma_start(bdram[:], zt[:])

        # gs stream: 8 chunks of [128, 4, 900] fp8 on the sync queue, each in
        # two half-DMAs so the first T-reduce matmul starts half a chunk early;
        # rt halves 2,3 ride behind it (needed only by groups 8+)
        gs_tiles = []
        for s in range(NCHUNK):
            gt = gs_pool.tile([128, 4, NN], FP8, tag="gt", name=f"gt{s}")
            src = gs8[512 * s : 512 * (s + 1)].rearrange("(p c) m -> p c m", c=4)
            nc.sync.dma_start(gt[:, 0:2, :], src[:, 0:2, :])
            nc.sync.dma_start(gt[:, 2:4, :], src[:, 2:4, :])
            gs_tiles.append(gt)
        for h4 in range(2, 4):
            cs = h4 * 512
            nc.sync.dma_start(
                rt_all[:, :, cs : cs + 512],
                rtp[:, cs : cs + 512].rearrange("(p c) m -> p c m", c=4),
            )

        logits_p = lgp_pool.tile([GPB * C, NG], F32, tag="logits")
        bd_tiles = {}

        def emit_treduce(s):
            # fp8 DoubleRow: K=256 per matmul (two c-columns), each batch
            # duplicated on 2 out rows (M=16; M=8 trips lw_dual_fp8 ISA check)
            adjp_t = adjp_pool.tile([2 * BPC, NN], F32, tag="adjp")
            for h in range(2):
                for lo, hi in ((0, 512), (512, NN)):
                    nc.tensor.matmul(
                        adjp_t[:, lo:hi], ones16_sb[:],
                        gs_tiles[s][:, 2 * h : 2 * h + 2, lo:hi],
                        start=(h == 0), stop=(h == 1),
                        perf_mode=mybir.MatmulPerfMode.DoubleRow,
                    )
            adjs_t = adjs_pool.tile([2 * BPC, NN], BF16, tag="adjs")
            nc.vector.tensor_copy(adjs_t[:, 0:300], adjp_t[:, 0:300])
            nc.scalar.copy(adjs_t[:, 300:NN], adjp_t[:, 300:NN])
            # scatter-write the two groups' diag blocks into the DRAM scratch
            adjs_ev = adjs_t[:].rearrange("(b r) m -> b r m", r=2)
            for half, eng in ((0, nc.gpsimd), (1, nc.scalar)):
                g = 2 * s + half
                wdst = bass.AP(
                    bdram, NB * g, [[PB * BDW + N, GPB], [BDW, N], [1, N]]
                )
                eng.dma_start(wdst, adjs_ev[4 * half : 4 * half + 4, 0, :])
            # read back both groups as one [128, 2, 120] block-diag tile
            bd_t = bd_pool.tile([128, 2, NB], BF16, tag="bd", name=f"bd{s}")
            nc.scalar.dma_start(
                bd_t[:],
                bdram[:, 2 * NB * s : 2 * NB * (s + 1)].rearrange(
                    "p (h m) -> p h m", h=2
                ),
            )
            bd_tiles[s] = bd_t

        ns_tiles = {}
        xwb_tiles = {}

        def emit_xw(s):
            # xw depends only on rt/wt -- hoisted early; xwb persists per pair.
            # PSUM->SBUF copies split across DVE/ACT for faster turnaround.
            xwp_t = xwp_pool.tile([128, 2, D], F32, tag="xwp")
            xwb_t = xwb_pool.tile([128, 2, D], BF16, tag="xwb", name=f"xwb{s}")
            xwb_tiles[s] = xwb_t
            for h in range(2):
                g = 2 * s + h
                for c4 in range(4):
                    nc.tensor.matmul(
                        xwp_t[:, h, :], rt_all[:, c4, 128 * g : 128 * (g + 1)],
                        wt_sb[:, c4, :], start=(c4 == 0), stop=(c4 == 3),
                    )
                nc.vector.tensor_copy(xwb_t[:, h, :], xwp_t[:, h, :])

        def emit_agg(s):
            # groups 2s, 2s+1 share one PSUM pair tile
            aggp_t = aggp_pool.tile([NB, 2, D], F32, tag="aggp")
            ns2_t = ns_pool.tile([NB, 2], F32, tag="ns", name=f"ns{s}")
            ns_tiles[s] = ns2_t
            nc.tensor.matmul(
                aggp_t[:], ones1_sb[:], cb2_sb[:],
                start=True, stop=False, skip_group_check=True,
            )
            for h in range(2):
                nc.tensor.matmul(
                    aggp_t[:, h, :], bd_tiles[s][:, h, :], xwb_tiles[s][:, h, :],
                    start=False, stop=(h == 1), skip_group_check=True,
                )
            # relu + pool-weight mult + free-dim sum fused: one DVE pass each
            for h in range(2):
                scr_t = scr_pool.tile([NB, D], F32, tag="scr")
                nc.vector.scalar_tensor_tensor(
                    scr_t[:], aggp_t[:, h, :], 0.0, pwb_sb[:],
                    mybir.AluOpType.max, mybir.AluOpType.mult,
                    accum_out=ns2_t[:, h : h + 1],
                )

        def emit_head(s):
            nc.tensor.matmul(
                logits_p[:, 2 * s : 2 * s + 2], hw_sb[:], ns_tiles[s][:],
                start=True, stop=True,
            )

        # software-pipelined: xw hoisted (no gs dependency), bd(s) consumed
        # one chunk after its read, heads one pair later
        emit_treduce(0)
        emit_treduce(1)
        for s in range(2, NCHUNK):
            emit_treduce(s)
            emit_xw(s - 2)
            if s >= 3:
                emit_agg(s - 3)
            if s >= 4:
                emit_head(s - 4)
        emit_xw(NCHUNK - 2)
        emit_agg(NCHUNK - 3)
        emit_head(NCHUNK - 4)
        emit_xw(NCHUNK - 1)
        emit_agg(NCHUNK - 2)
        emit_head(NCHUNK - 3)
        emit_agg(NCHUNK - 1)
        emit_head(NCHUNK - 2)
        emit_head(NCHUNK - 1)

        # softmax over the 7 classes (partition sub-blocks of 7)
        e_t = tail_pool.tile([GPB * C, NG], F32, tag="e")
        nc.scalar.activation(
            e_t[:], logits_p[:], mybir.ActivationFunctionType.Exp, bias=hbb_sb
        )
        sum_p = xwp_pool.tile([GPB, NG], F32, tag="xwp", name="sum_p")
        nc.tensor.matmul(sum_p[:], b7_sb[:], e_t[:], start=True, stop=True)
        ssb_t = tail_pool.tile([GPB, NG], F32, tag="ssb")
        nc.vector.tensor_copy(ssb_t[:], sum_p[:])
        bcast_p = aggp_pool.tile([GPB * C, NG], F32, tag="aggp", name="bcast_p")
        nc.tensor.matmul(bcast_p[:], b7t_sb[:], ssb_t[:], start=True, stop=True)
        rs_t = tail_pool.tile([GPB * C, NG], F32, tag="rs")
        nc.vector.reciprocal(rs_t[:], bcast_p[:])
        res_t = tail_pool.tile([GPB * C, NG], F32, tag="res")
        nc.vector.tensor_mul(res_t[:], e_t[:], rs_t[:])
        nc.sync.dma_start(out.rearrange("(g bi) c -> (bi c) g", bi=GPB), res_t[:])

    nc.compile()
    return nc


_NC_CACHE = None


def _get_nc():
    global _NC_CACHE
    if _NC_CACHE is None:
        _NC_CACHE = _build_nc()
    return _NC_CACHE


def _f32c(x):
    return np.ascontiguousarray(np.asarray(x, dtype=np.float32))


def _prepare_in_maps(real, graph_sigs, W, conv_bias, pool_w, pool_b, head_w, head_b):
    real = _f32c(real)
    graph_sigs = _f32c(graph_sigs)
    W = _f32c(W)
    head_w = _f32c(head_w)

    wt = W.reshape(128, 4, D).astype(NP_BF16)  # wt[p, c, :] = W[4p+c]
    # fp8 DoubleRow T-reduce weights: batch m//2 at partitions [16b, 16b+16)
    ones16 = np.zeros((128, 2, 16), dtype=np.float32)
    for m in range(16):
        b = m // 2
        ones16[16 * b : 16 * (b + 1), :, m] = np.float32(1.0 / T)
    ones16 = ones16.astype(NP_FP8)
    # bf16 blob: ones1 | conv_bias x2
    cbh = np.zeros((128, 648), dtype=NP_BF16)
    cbh[0, BPC : BPC + NB] = np.float32(1.0)
    cbh[0, 136 : 136 + D] = _f32c(conv_bias).astype(NP_BF16)
    cbh[0, 136 + D : 136 + 2 * D] = _f32c(conv_bias).astype(NP_BF16)
    # f32 blob: pwb | hwblk | hbb | b7 | b7t
    cf = np.zeros((128, 320), dtype=np.float32)
    cf[:, 0:D] = _f32c(pool_w)[None, :]
    # hwblk rows 30k+n -> col k*7+c = head_w[c, n]
    for k in range(GPB):
        cf[N * k : N * (k + 1), D + C * k : D + C * (k + 1)] = head_w.T
    # pool_b shifts every node score by a constant; fold into the head bias
    hb_eff = _f32c(head_b) + np.float32(np.asarray(pool_b)) * head_w.sum(axis=1)
    cf[0 : GPB * C, 284] = np.tile(hb_eff, GPB)
    for k in range(GPB):
        cf[C * k : C * (k + 1), 285 + k] = 1.0
    for k in range(GPB):
        cf[0:GPB, 289 + C * k : 289 + C * (k + 1)] = np.eye(GPB)[:, k : k + 1]

    consts = {"wt": wt, "cbh": cbh, "cf": cf, "ones16": ones16}
    in_maps = []
    for c in range(NCORES):
        s = slice(c * BL, (c + 1) * BL)
        gs8 = graph_sigs[s].reshape(BL * T, NN).astype(NP_FP8)
        rt = real[s].transpose(2, 0, 1).reshape(F_IN, NG, GPB, N)
        rtp = np.zeros((F_IN, NG, GPB, PB), dtype=NP_BF16)
        rtp[:, :, :, 0:N] = rt.astype(NP_BF16)
        in_maps.append(
            {
                "gs8": gs8,
                "rtp": np.ascontiguousarray(rtp.reshape(F_IN, NG * 128)),
                **consts,
            }
        )
    return in_maps


def kernel(real, imag, graph_sigs, W, conv_bias, pool_w, pool_b, head_w, head_b):
    del imag  # unused by the forward pass
    in_maps = _prepare_in_maps(
        real, graph_sigs, W, conv_bias, pool_w, pool_b, head_w, head_b
    )
    nc = _get_nc()
    res = run_bass_kernel_spmd(nc, in_maps, core_ids=list(range(NCORES)))
    return np.concatenate([res.results[c]["out"] for c in range(NCORES)], axis=0)
